# revision 17
# baseline (speedup 1.0000x reference)
"""Trainium2 Bass kernel for nn_AttentionAggregator (GNN message passing).

Math (per batch row b, with N=64 neighbors, F=128 in-features, H=8 heads, D=64):
    lin  = x @ W_lin                                  [B, N, 512]
    att  = lin[:,0,:] @ W_att[:512] + lin @ W_att[512:]   [B, N, 8]
    att  = LeakyReLU_0.2(att); masked softmax over N per (b, h)
    out  = relu(lin * aw)                             [B, N, 512]

Key algebraic refactor: att terms contract through W_lin, so
    lin @ W_att[512:]  == x @ (W_lin @ W_att[512:])   (precomputed [128, 8])
    src @ W_att[:512]  == x[:,0,:] @ (W_lin @ W_att[:512])
which lets one PE matmul per 128-row tile produce lin (512 cols) and both
attention terms (16 cols) from the same stationary x^T tile.  Cross-partition
softmax sum / broadcast are done with constant selector / block-diagonal
matmuls on the PE (no GPSIMD, no transposes of activations).

Sharding: pure data-parallel over batch: 512 batch rows (= 256 128-row tiles)
per NeuronCore, weights replicated.  Host pre-transposes x tiles to [F, rows]
(the PE needs the contraction dim on partitions) and pre-bakes the mask into
an additive bias (0 / -1e30) consumed by the fused exp activation.
"""

import os
from contextlib import ExitStack

import numpy as np

import concourse.bacc as bacc
import concourse.bass as bass
import concourse.tile as tile
from concourse import mybir
from concourse.bass_utils import run_bass_kernel_spmd
from concourse.tile_rust import add_dep_helper

B, N, F = 4096, 64, 128
H, D = 8, 64
HD = H * D  # 512
NCORES = 8
BSHARD = B // NCORES  # 512
ROWS = BSHARD * N  # 32768
TILES = ROWS // 128  # 256

f32 = mybir.dt.float32
f32r = mybir.dt.float32r

LAST_RESULT = None  # test harness reads exec_time_ns / trace from here


def build_nc(tiles: int = TILES) -> bass.Bass:
    # Bacc (not raw Bass): its compile() runs the legalization walrus
    # requires — move_matmul_waits_to_ldweights, generate_event_semaphores
    # (max 1 wait per instruction), insert_act_table_loads.
    nc = bacc.Bacc("TRN2", target_bir_lowering=False, debug=False)
    rows = tiles * 128

    xt = nc.declare_dram_parameter("xt", [tiles, 128, 128], f32, isOutput=False)
    wbig = nc.declare_dram_parameter("wbig", [128, HD + 16], f32, isOutput=False)
    sel = nc.declare_dram_parameter("sel", [128, 128], f32, isOutput=False)
    bdiag = nc.declare_dram_parameter("bdiag", [128, 128], f32, isOutput=False)
    maskb = nc.declare_dram_parameter("maskb", [128, tiles], f32, isOutput=False)
    out = nc.declare_dram_parameter("out", [rows, HD], f32, isOutput=True)

    mult = mybir.AluOpType.mult
    mmax = mybir.AluOpType.max

    with tile.TileContext(nc) as tc, ExitStack() as ctx:
        consts = ctx.enter_context(tc.tile_pool(name="consts", bufs=1))
        xin = ctx.enter_context(tc.tile_pool(name="xin", bufs=4))
        outp = ctx.enter_context(tc.tile_pool(name="outp", bufs=4))
        small = ctx.enter_context(tc.tile_pool(name="small", bufs=4))
        plin = ctx.enter_context(tc.tile_pool(name="plin", bufs=2, space="PSUM"))
        patt = ctx.enter_context(tc.tile_pool(name="patt", bufs=2, space="PSUM"))
        pattb = ctx.enter_context(tc.tile_pool(name="pattb", bufs=2, space="PSUM"))
        pden = ctx.enter_context(tc.tile_pool(name="pden", bufs=2, space="PSUM"))

        # Walrus allows at most ONE semaphore wait per Matmult (the sync
        # lands on the LDWEIGHTS slot), and Tile's vector clocks are not
        # transitive across engines. So every tensor a matmul reads is
        # staged through a DVE copy: PE then only ever waits on the DVE
        # semaphore (plus ACT for the attention chain), never on DMA lanes.
        # W_lin is float32r-typed (bit-identical to f32; the PE rounds
        # internally) so the BIR verifier sees an f32r producer for the
        # full-rate f32r matmul. The attention columns stay exact fp32.
        wlin_raw = consts.tile([128, HD], f32r)
        nc.gpsimd.dma_start(out=wlin_raw, in_=wbig[:, 0:HD].bitcast(f32r))
        wlin_sb = consts.tile([128, HD], f32r)
        nc.vector.tensor_copy(out=wlin_sb, in_=wlin_raw)
        watt_raw = consts.tile([128, 16], f32)
        nc.gpsimd.dma_start(out=watt_raw, in_=wbig[:, HD : HD + 16])
        watt_sb = consts.tile([128, 16], f32)
        nc.vector.tensor_copy(out=watt_sb, in_=watt_raw)
        sel_raw = consts.tile([128, 128], f32)
        nc.gpsimd.dma_start(out=sel_raw, in_=sel[:])
        sel_sb = consts.tile([128, 128], f32)
        nc.vector.tensor_copy(out=sel_sb, in_=sel_raw)
        bd_raw = consts.tile([128, 128], f32)
        nc.gpsimd.dma_start(out=bd_raw, in_=bdiag[:])
        bd_sb = consts.tile([128, 128], f32)
        nc.vector.tensor_copy(out=bd_sb, in_=bd_raw)
        maskb_sb = consts.tile([128, tiles], f32)
        nc.gpsimd.dma_start(out=maskb_sb, in_=maskb[:])

        prev_stt = None
        for t in range(tiles):
            # x^T tile: [F=128 partitions, 128 rows] (pre-transposed on host).
            # DMA lands in xr_sb; the DVE copy to x_sb is what the matmuls
            # depend on, so they wait on DVE instead of a DMA lane.
            xr_sb = xin.tile([128, 128], f32r, tag="xr")
            nc.gpsimd.dma_start(out=xr_sb, in_=xt[t].bitcast(f32r))
            x_sb = xin.tile([128, 128], f32r, tag="x")
            xcopy = nc.vector.tensor_copy(out=x_sb, in_=xr_sb)
            if prev_stt is not None:
                # Order the copy after the previous tile's final DVE op so a
                # matmul's single DVE wait (on this copy) also covers every
                # PSUM-slot release up through tile t-1.
                add_dep_helper(
                    xcopy.ins,
                    prev_stt.ins,
                    sync=False,
                    reason="serialize DVE stream for single-wait matmuls",
                )

            # attB = x @ (W_lin@W_att1): rows 0 and 64 hold the per-batch src
            # term. Issued FIRST so it (not the lin matmul, which draws an
            # injected PE self-wait) carries the tile's single DVE data-wait.
            attb_ps = pattb.tile([128, 8], f32, tag="attb")
            nc.tensor.matmul(
                attb_ps, x_sb.bitcast(f32), watt_sb[:, 8:16], start=True, stop=True
            )
            # lin = x @ W_lin  -> [rows, 512]  (fp32r full-rate, N=512)
            lin_ps = plin.tile([128, HD], f32, tag="lin")
            nc.tensor.matmul(lin_ps, x_sb, wlin_sb, start=True, stop=True)
            attb_sb = small.tile([128, 8], f32, tag="attb")
            nc.scalar.copy(out=attb_sb, in_=attb_ps)
            # att = attA + sel.T @ attB  (PE accumulation group on att_ps):
            # attA = x @ (W_lin@W_att2); the selector matmul broadcasts the
            # src term from rows {0, 64} to all 64 rows of each batch.
            att_ps = patt.tile([128, 8], f32, tag="att")
            nc.tensor.matmul(
                att_ps, x_sb.bitcast(f32), watt_sb[:, 0:8], start=True, stop=False
            )
            nc.tensor.matmul(att_ps, sel_sb, attb_sb, start=False, stop=True)
            # LeakyReLU(0.2): max(x, 0.2x)
            attl_sb = small.tile([128, 8], f32, tag="attl")
            nc.vector.tensor_scalar(
                out=attl_sb, in0=att_ps, scalar1=0.2, scalar2=None, op0=mult
            )
            nc.vector.tensor_max(attl_sb, attl_sb, att_ps)
            # masked exp: exp(att + maskbias), maskbias in {0, -1e30}
            ew_sb = small.tile([128, 8], f32, tag="ew")
            nc.scalar.activation(
                out=ew_sb,
                in_=attl_sb,
                func=mybir.ActivationFunctionType.Exp,
                bias=maskb_sb[:, t : t + 1],
                scale=1.0,
            )
            # denominator, broadcast to every row of its batch: blockdiag.T @ ew
            den_ps = pden.tile([128, 8], f32, tag="den")
            nc.tensor.matmul(den_ps, bd_sb, ew_sb, start=True, stop=True)
            rden_sb = small.tile([128, 8], f32, tag="rden")
            nc.vector.reciprocal(rden_sb, den_ps)
            aw_sb = small.tile([128, 8], f32, tag="aw")
            nc.vector.tensor_mul(aw_sb, ew_sb, rden_sb)
            # out = relu(lin) * aw  (fused: (lin max 0) * aw_broadcast)
            o_sb = outp.tile([128, HD], f32, tag="o")
            prev_stt = nc.vector.scalar_tensor_tensor(
                out=o_sb.rearrange("p (h d) -> p h d", h=H),
                in0=lin_ps.rearrange("p (h d) -> p h d", h=H),
                scalar=0.0,
                in1=aw_sb.to_broadcast([128, H, D]),
                op0=mmax,
                op1=mult,
            )
            nc.gpsimd.dma_start(out=out[t * 128 : (t + 1) * 128, :], in_=o_sb)

    nc.compile()
    return nc


def _host_weights(W_lin, W_att):
    W_lin64 = W_lin.astype(np.float64)
    wc2 = (W_lin64 @ W_att[HD:].astype(np.float64)).astype(np.float32)
    wc1 = (W_lin64 @ W_att[:HD].astype(np.float64)).astype(np.float32)
    wbig = np.ascontiguousarray(
        np.concatenate([W_lin.astype(np.float32), wc2, wc1], axis=1)
    )
    # sel.T @ attB broadcasts row 0 -> rows 0:64 and row 64 -> rows 64:128
    sel = np.zeros((128, 128), np.float32)
    sel[0, 0:64] = 1.0
    sel[64, 64:128] = 1.0
    # blockdiag.T @ ew gives each row the sum over its own 64-row batch
    bd = np.zeros((128, 128), np.float32)
    bd[0:64, 0:64] = 1.0
    bd[64:128, 64:128] = 1.0
    return wbig, sel, bd


def _core_inputs(x_shard, mask_shard, wbig, sel, bd):
    nb = x_shard.shape[0]
    tiles = nb * N // 128
    xt = np.ascontiguousarray(
        x_shard.reshape(tiles, 128, F).transpose(0, 2, 1)
    )
    mb_ = np.where(mask_shard.reshape(tiles, 128) != 0, 0.0, -1e30).astype(
        np.float32
    )
    maskb = np.ascontiguousarray(mb_.T)
    return {"xt": xt, "wbig": wbig, "sel": sel, "bdiag": bd, "maskb": maskb}


def kernel(x, W_lin, W_att, mask):
    global LAST_RESULT
    x = np.asarray(x, dtype=np.float32)
    W_lin = np.asarray(W_lin, dtype=np.float32)
    W_att = np.asarray(W_att, dtype=np.float32)
    mask = np.asarray(mask)

    wbig, sel, bd = _host_weights(W_lin, W_att)
    in_maps = []
    for c in range(NCORES):
        in_maps.append(
            _core_inputs(
                x[c * BSHARD : (c + 1) * BSHARD],
                mask[c * BSHARD : (c + 1) * BSHARD],
                wbig,
                sel,
                bd,
            )
        )

    nc = build_nc(TILES)
    trace = os.environ.get("KERNEL_TRACE", "0") == "1"
    tmpdir = os.environ.get("KERNEL_TRACE_DIR") or None
    res = run_bass_kernel_spmd(
        nc, in_maps, list(range(NCORES)), trace=trace, tmpdir=tmpdir
    )
    LAST_RESULT = res
    return np.concatenate(
        [res.results[c]["out"].reshape(BSHARD, N, HD) for c in range(NCORES)],
        axis=0,
    )


# revision 25
# speedup vs baseline: 2.5985x; 2.5985x over previous
"""Trainium2 Bass kernel for nn_AttentionAggregator (GNN message passing).

Math (per batch row b, with N=64 neighbors, F=128 in-features, H=8 heads, D=64):
    lin  = x @ W_lin                                      [B, N, 512]
    att  = lin[:,0,:] @ W_att[:512] + lin @ W_att[512:]   [B, N, 8]
    att  = LeakyReLU_0.2(att); masked softmax over N per (b, h)
    out  = relu(lin * aw)                                 [B, N, 512]

Key refactors:
  * The attention terms contract through W_lin:
        lin @ W_att[512:] == x @ (W_lin @ W_att[512:])    (wc2: [128, 8])
        src @ W_att[:512] == x[:,0,:] @ (W_lin @ W_att[:512])  (wc1)
    so one tiny constant-stationary matmul (watt16 = [wc2 | wc1]) computes
    both from x^T directly, TRANSPOSED: attT[16, rows] = watt16.T @ x^T.
    In this layout the softmax axis (n) is the free dim: the src-term
    broadcast and the 1/den broadcast are free-dim step-0 APs, and the
    denominator falls out of the exp activation's accum_out. No selector
    matmuls, no cross-partition reductions.
  * The mask lands in logit space BEFORE LeakyReLU via a rank-1 bf16
    matmul accumulate (ones[1,8] x maskrow[1,rows] of {0,-1e30}):
    exp(leaky(att-1e30)) == 0 exactly, same as the reference's post-leaky
    -1e9. Only attA rows 0:8 get it; the src rows 8:16 stay unmasked.
  * lin runs as float32r (full PE rate at free dim >= 256, ~1e-4 rel err).
  * aw[8, rows] is cast to bf16 and PE-transposed back to row layout for
    the final fused (relu(lin) * aw) DVE pass.

Sharding: pure data-parallel over batch: 512 batch rows per core
(128 double-tiles of 256 rows), weights replicated. Host pre-transposes
x into [F, rows] tiles and pre-bakes the mask row vector.
"""

import os
from contextlib import ExitStack

import ml_dtypes
import numpy as np

import concourse.bacc as bacc
import concourse.bass as bass
import concourse.tile as tile
from concourse import mybir
from concourse.bass_utils import run_bass_kernel_spmd

B, N, F = 4096, 64, 128
H, D = 8, 64
HD = H * D  # 512
NCORES = 8
BSHARD = B // NCORES  # 512
ROWS = BSHARD * N  # 32768
DT_ROWS = 256  # rows per double-tile (4 batches)
DTILES = ROWS // DT_ROWS  # 128

f32 = mybir.dt.float32
f32r = mybir.dt.float32r
bf16 = mybir.dt.bfloat16
f16 = mybir.dt.float16

LAST_RESULT = None  # test harness reads exec_time_ns / trace from here


def build_nc(dtiles: int = DTILES) -> bass.Bass:
    # Bacc (not raw Bass): its compile() runs the legalization walrus
    # requires — move_matmul_waits_to_ldweights, generate_event_semaphores
    # (max 1 wait per instruction), insert_act_table_loads.
    nc = bacc.Bacc("TRN2", target_bir_lowering=False, debug=False)
    rows = dtiles * DT_ROWS

    xt = nc.declare_dram_parameter("xt", [dtiles, F, DT_ROWS], f32, isOutput=False)
    wlin_d = nc.declare_dram_parameter("wlin", [F, HD], f32, isOutput=False)
    watt_d = nc.declare_dram_parameter("watt", [F, 16], f32, isOutput=False)
    ident_d = nc.declare_dram_parameter("ident8", [8, 8], f16, isOutput=False)
    maskrow_d = nc.declare_dram_parameter("maskrow", [1, rows], bf16, isOutput=False)
    out = nc.declare_dram_parameter("out", [rows, HD], f32, isOutput=True)

    mult = mybir.AluOpType.mult
    mmax = mybir.AluOpType.max

    with tile.TileContext(nc) as tc, ExitStack() as ctx:
        consts = ctx.enter_context(tc.tile_pool(name="consts", bufs=1))
        xin = ctx.enter_context(tc.tile_pool(name="xin", bufs=3))
        outp = ctx.enter_context(tc.tile_pool(name="outp", bufs=3))
        small = ctx.enter_context(tc.tile_pool(name="small", bufs=3))
        plin = ctx.enter_context(tc.tile_pool(name="plin", bufs=4, space="PSUM"))
        patt = ctx.enter_context(tc.tile_pool(name="patt", bufs=2, space="PSUM"))
        paw = ctx.enter_context(tc.tile_pool(name="paw", bufs=2, space="PSUM"))

        wlin_sb = consts.tile([F, HD], f32r)
        nc.sync.dma_start(out=wlin_sb, in_=wlin_d[:].bitcast(f32r))
        watt_sb = consts.tile([F, 16], f32r)
        nc.sync.dma_start(out=watt_sb, in_=watt_d[:].bitcast(f32r))
        ident_sb = consts.tile([8, 8], f16)
        nc.sync.dma_start(out=ident_sb, in_=ident_d[:])
        maskrow_sb = consts.tile([1, rows], bf16)
        nc.sync.dma_start(out=maskrow_sb, in_=maskrow_d[:])
        mones_sb = consts.tile([1, 8], bf16)
        nc.vector.memset(mones_sb, 1.0)

        for t in range(dtiles):
            # x^T double tile: [F=128 partitions, 256 rows] (host transposed)
            x_sb = xin.tile([F, DT_ROWS], f32r, tag="x")
            nc.sync.dma_start(out=x_sb, in_=xt[t].bitcast(f32r))

            # attT[8, 512] = [attA cols 0:256 | attB cols 256:512], both at
            # partition base 0 (DVE/ACT reads can't start at partition 8).
            # The rank-1 bf16 mask accumulate then puts -1e30 on masked attA
            # entries (pre-leaky logit masking); src cols stay unmasked.
            attT_ps = patt.tile([8, 2 * DT_ROWS], f32, tag="attT")
            nc.tensor.matmul(
                attT_ps[:, 0:DT_ROWS], watt_sb[:, 0:8], x_sb, start=True, stop=False
            )
            nc.tensor.matmul(
                attT_ps[:, DT_ROWS : 2 * DT_ROWS],
                watt_sb[:, 8:16],
                x_sb,
                start=False,
                stop=True,
            )
            nc.tensor.matmul(
                attT_ps[:, 0:DT_ROWS],
                mones_sb,
                maskrow_sb[:, t * DT_ROWS : (t + 1) * DT_ROWS],
                start=False,
                stop=True,
                skip_group_check=True,
            )

            # lin[rows, 512] in row layout for the contiguous output DMA
            lin_a = plin.tile([128, HD], f32, tag="lin")
            nc.tensor.matmul(lin_a, x_sb[:, 0:128], wlin_sb, start=True, stop=True)
            lin_b = plin.tile([128, HD], f32, tag="lin")
            nc.tensor.matmul(lin_b, x_sb[:, 128:256], wlin_sb, start=True, stop=True)

            # ACT stages attT to SBUF once the accumulate group closes
            attT_sb = small.tile([8, 2 * DT_ROWS], f32, tag="attT_sb")
            nc.scalar.copy(out=attT_sb, in_=attT_ps)

            # att = attA + src-term broadcast (free-dim step-0 AP over n);
            # the src term lives at attT_sb[:, 256 + 64*b]: view [8, 4, 1]
            attS_sb = small.tile([8, 4, N], f32, tag="attS")
            in1 = attT_sb[:, DT_ROWS : 2 * DT_ROWS].rearrange(
                "h (b n) -> h b n", b=4
            )[:, :, 0:1]
            nc.vector.tensor_tensor(
                out=attS_sb,
                in0=attT_sb[:, 0:DT_ROWS].rearrange("h (b n) -> h b n", b=4),
                in1=in1.to_broadcast([8, 4, N]),
                op=mybir.AluOpType.add,
            )
            # LeakyReLU(0.2): (x*0.2) max x
            attL_sb = small.tile([8, DT_ROWS], f32, tag="attL")
            nc.vector.scalar_tensor_tensor(
                out=attL_sb.rearrange("h (b n) -> h b n", b=4),
                in0=attS_sb,
                scalar=0.2,
                in1=attS_sb,
                op0=mult,
                op1=mmax,
            )
            # exp per batch; accum_out gives the softmax denominator
            ewT_sb = small.tile([8, DT_ROWS], f32, tag="ewT")
            denT_sb = small.tile([8, 4], f32, tag="denT")
            for b4 in range(4):
                nc.scalar.activation(
                    out=ewT_sb[:, b4 * N : (b4 + 1) * N],
                    in_=attL_sb[:, b4 * N : (b4 + 1) * N],
                    func=mybir.ActivationFunctionType.Exp,
                    accum_out=denT_sb[:, b4 : b4 + 1],
                )
            rden_sb = small.tile([8, 4], f32, tag="rden")
            nc.vector.reciprocal(rden_sb, denT_sb)
            # aw = ew / den (bf16 out for the cheap PE transpose)
            awT_sb = small.tile([8, DT_ROWS], f16, tag="awT")
            nc.vector.tensor_tensor(
                out=awT_sb.rearrange("h (b n) -> h b n", b=4),
                in0=ewT_sb.rearrange("h (b n) -> h b n", b=4),
                in1=rden_sb[:, :, None].to_broadcast([8, 4, N]),
                op=mult,
            )
            # transpose aw back to row layout: [8, 256] -> 2 x [128, 8]
            aw_ps = paw.tile([128, 16], f16, tag="aw_ps")
            nc.tensor.transpose(aw_ps[:, 0:8], awT_sb[:, 0:128], ident_sb)
            nc.tensor.transpose(aw_ps[:, 8:16], awT_sb[:, 128:256], ident_sb)
            aw_sb = small.tile([128, 16], f16, tag="aw_sb")
            nc.scalar.copy(out=aw_sb, in_=aw_ps)

            # out = relu(lin) * aw, fused on DVE; both 128-row halves into
            # one [128, 1024] tile -> single 512KB output DMA
            o_sb = outp.tile([128, 2, HD], f32, tag="o")
            nc.vector.scalar_tensor_tensor(
                out=o_sb[:, 0, :].rearrange("p (h d) -> p h d", h=H),
                in0=lin_a.rearrange("p (h d) -> p h d", h=H),
                scalar=0.0,
                in1=aw_sb[:, 0:8].to_broadcast([128, H, D]),
                op0=mmax,
                op1=mult,
            )
            nc.vector.scalar_tensor_tensor(
                out=o_sb[:, 1, :].rearrange("p (h d) -> p h d", h=H),
                in0=lin_b.rearrange("p (h d) -> p h d", h=H),
                scalar=0.0,
                in1=aw_sb[:, 8:16].to_broadcast([128, H, D]),
                op0=mmax,
                op1=mult,
            )
            out_view = out[t * DT_ROWS : (t + 1) * DT_ROWS, :].rearrange(
                "(two p) hd -> p two hd", two=2
            )
            nc.sync.dma_start(out=out_view, in_=o_sb)

    nc.compile()
    return nc


def _host_weights(W_lin, W_att):
    W_lin64 = W_lin.astype(np.float64)
    wc2 = (W_lin64 @ W_att[HD:].astype(np.float64)).astype(np.float32)
    wc1 = (W_lin64 @ W_att[:HD].astype(np.float64)).astype(np.float32)
    watt16 = np.ascontiguousarray(np.concatenate([wc2, wc1], axis=1))  # [128, 16]
    ident8 = np.eye(8, dtype=np.float16)
    return np.ascontiguousarray(W_lin.astype(np.float32)), watt16, ident8


def _core_inputs(x_shard, mask_shard, wlin, watt16, ident8):
    nb = x_shard.shape[0]
    dtiles = nb * N // DT_ROWS
    xtv = np.ascontiguousarray(
        x_shard.reshape(dtiles, DT_ROWS, F).transpose(0, 2, 1)
    )
    mrow = np.where(mask_shard.reshape(1, -1) != 0, 0.0, -1e30).astype(
        ml_dtypes.bfloat16
    )
    return {
        "xt": xtv,
        "wlin": wlin,
        "watt": watt16,
        "ident8": ident8,
        "maskrow": mrow,
    }


def kernel(x, W_lin, W_att, mask):
    global LAST_RESULT
    x = np.asarray(x, dtype=np.float32)
    W_lin = np.asarray(W_lin, dtype=np.float32)
    W_att = np.asarray(W_att, dtype=np.float32)
    mask = np.asarray(mask)

    wlin, watt16, ident8 = _host_weights(W_lin, W_att)
    in_maps = []
    for c in range(NCORES):
        in_maps.append(
            _core_inputs(
                x[c * BSHARD : (c + 1) * BSHARD],
                mask[c * BSHARD : (c + 1) * BSHARD],
                wlin,
                watt16,
                ident8,
            )
        )

    nc = build_nc(DTILES)
    trace = os.environ.get("KERNEL_TRACE", "0") == "1"
    tmpdir = os.environ.get("KERNEL_TRACE_DIR") or None
    res = run_bass_kernel_spmd(
        nc, in_maps, list(range(NCORES)), trace=trace, tmpdir=tmpdir
    )
    LAST_RESULT = res
    return np.concatenate(
        [res.results[c]["out"].reshape(BSHARD, N, HD) for c in range(NCORES)],
        axis=0,
    )


# revision 29
# speedup vs baseline: 2.8515x; 1.0974x over previous
"""Trainium2 Bass kernel for nn_AttentionAggregator (GNN message passing).

Math (per batch row b, with N=64 neighbors, F=128 in-features, H=8 heads, D=64):
    lin  = x @ W_lin                                      [B, N, 512]
    att  = lin[:,0,:] @ W_att[:512] + lin @ W_att[512:]   [B, N, 8]
    att  = LeakyReLU_0.2(att); masked softmax over N per (b, h)
    out  = relu(lin * aw)                                 [B, N, 512]

Design (v3, tuned against neuron-profile traces):
  * Attention contracts through W_lin (wc = W_lin @ W_att blocks, [128, 16])
    and is computed TRANSPOSED per 256-row tile: attT[8, 512] =
    [wcA.T @ xT | wcB.T @ xT], so the softmax axis (n) is a free dim:
    src-term/den broadcasts are step-0 APs and no cross-partition reduction
    or selector matmuls exist.
  * Mask is pre-baked on the host to {0, -1e30}, injected in LOGIT space
    before LeakyReLU via a rank-1 bf16 matmul accumulate onto attA
    (exp(leaky(att - 1e30)) == 0 == the reference's post-leaky -1e9 path).
    Src columns stay unmasked (reference uses slot 0 regardless of mask).
  * fp16 matmul inputs (x, W_lin, wc): 1 cycle/row on the PE (fp32 runs at
    ~4 cyc/row) and half the input DMA traffic. PSUM accumulation is fp32;
    end-to-end rel err ~5e-4.
  * fp16 OUTPUT DMA (host upcasts to f32): halves the dominant 536MB
    output traffic.
  * lin_a/lin_b live in one [128, 1024] PSUM tile (2 banks) so a single
    fused DVE scalar_tensor_tensor computes relu(lin)*aw per 256 rows.
  * The small attention-chain ops are batched over QUAD=4 tiles ([8, 1024]
    slabs) to amortize per-instruction overhead, and are spread across
    GpSimd (adds/leaky/awmul), ACT (copies/exp), DVE (reduce/recip/out).

Sharding: pure data-parallel over batch: 512 batch rows per core
(128 tiles of 256 rows), weights replicated.
"""

import os
from contextlib import ExitStack

import ml_dtypes
import numpy as np

import concourse.bacc as bacc
import concourse.bass as bass
import concourse.tile as tile
from concourse import mybir
from concourse.bass_utils import run_bass_kernel_spmd

B, N, F = 4096, 64, 128
H, D = 8, 64
HD = H * D  # 512
NCORES = 8
BSHARD = B // NCORES  # 512
ROWS = BSHARD * N  # 32768
DT_ROWS = 256  # rows per tile (4 batch elements)
DTILES = ROWS // DT_ROWS  # 128
QUAD = 4  # attention-chain batching factor

f32 = mybir.dt.float32
bf16 = mybir.dt.bfloat16
f16 = mybir.dt.float16

LAST_RESULT = None  # test harness reads exec_time_ns / trace from here


def build_nc(dtiles: int = DTILES) -> bass.Bass:
    nc = bacc.Bacc("TRN2", target_bir_lowering=False, debug=False)
    rows = dtiles * DT_ROWS
    assert dtiles % QUAD == 0

    xt = nc.declare_dram_parameter("xt", [dtiles, F, DT_ROWS], f16, isOutput=False)
    wlin_d = nc.declare_dram_parameter("wlin", [F, HD], f16, isOutput=False)
    watt_d = nc.declare_dram_parameter("watt", [F, 16], f16, isOutput=False)
    ident_d = nc.declare_dram_parameter("ident8", [8, 8], f16, isOutput=False)
    maskrow_d = nc.declare_dram_parameter("maskrow", [1, rows], bf16, isOutput=False)
    out = nc.declare_dram_parameter("out", [rows, HD], f16, isOutput=True)

    mult = mybir.AluOpType.mult
    mmax = mybir.AluOpType.max

    with tile.TileContext(nc) as tc, ExitStack() as ctx:
        consts = ctx.enter_context(tc.tile_pool(name="consts", bufs=1))
        xin = ctx.enter_context(tc.tile_pool(name="xin", bufs=3))
        outp = ctx.enter_context(tc.tile_pool(name="outp", bufs=3))
        small = ctx.enter_context(tc.tile_pool(name="small", bufs=3))
        slabs = ctx.enter_context(tc.tile_pool(name="slabs", bufs=2))
        plin = ctx.enter_context(tc.tile_pool(name="plin", bufs=2, space="PSUM"))
        patt = ctx.enter_context(tc.tile_pool(name="patt", bufs=2, space="PSUM"))
        paw = ctx.enter_context(tc.tile_pool(name="paw", bufs=2, space="PSUM"))

        wlin_sb = consts.tile([F, HD], f16)
        nc.sync.dma_start(out=wlin_sb, in_=wlin_d[:])
        watt_sb = consts.tile([F, 16], f16)
        nc.sync.dma_start(out=watt_sb, in_=watt_d[:])
        ident_sb = consts.tile([8, 8], f16)
        nc.sync.dma_start(out=ident_sb, in_=ident_d[:])
        maskrow_sb = consts.tile([1, rows], bf16)
        nc.sync.dma_start(out=maskrow_sb, in_=maskrow_d[:])
        mones_sb = consts.tile([1, 8], bf16)
        nc.vector.memset(mones_sb, 1.0)

        def tile_front(t, x_sl):
            """Matmuls + attT staging for tile t; x_sl = [F, 256] fp16."""
            # attT[8, 512] = [attA | attB] at partition base 0; the rank-1
            # bf16 accumulate puts -1e30 on masked attA logits.
            attT_ps = patt.tile([8, 2 * DT_ROWS], f32, tag="attT")
            nc.tensor.matmul(
                attT_ps[:, 0:DT_ROWS], watt_sb[:, 0:8], x_sl, start=True, stop=False
            )
            nc.tensor.matmul(
                attT_ps[:, DT_ROWS:], watt_sb[:, 8:16], x_sl, start=False, stop=True
            )
            nc.tensor.matmul(
                attT_ps[:, 0:DT_ROWS],
                mones_sb,
                maskrow_sb[:, t * DT_ROWS : (t + 1) * DT_ROWS],
                start=False,
                stop=True,
                skip_group_check=True,
            )
            # lin for both 128-row halves into one 2-bank PSUM tile
            lin_ps = plin.tile([128, 2, HD], f32, tag="lin")
            nc.tensor.matmul(
                lin_ps[:, 0, :], x_sl[:, 0:128], wlin_sb, start=True, stop=True
            )
            nc.tensor.matmul(
                lin_ps[:, 1, :], x_sl[:, 128:256], wlin_sb, start=True, stop=True
            )
            return attT_ps, lin_ps

        def tile_back(t, lin_ps, awT_slab, islab):
            """aw transpose + fused output + store for tile t."""
            aw_ps = paw.tile([128, 16], f16, tag="aw_ps")
            base = islab * DT_ROWS
            nc.tensor.transpose(
                aw_ps[:, 0:8], awT_slab[:, base : base + 128], ident_sb
            )
            nc.tensor.transpose(
                aw_ps[:, 8:16], awT_slab[:, base + 128 : base + 256], ident_sb
            )
            aw_sb = small.tile([128, 16], f16, tag="aw_sb")
            nc.scalar.copy(out=aw_sb, in_=aw_ps)

            o_sb = outp.tile([128, 2, HD], f16, tag="o")
            nc.vector.scalar_tensor_tensor(
                out=o_sb.rearrange("p two (h d) -> p (two h) d", h=H),
                in0=lin_ps.rearrange("p two (h d) -> p (two h) d", h=H),
                scalar=0.0,
                in1=aw_sb.to_broadcast([128, 2 * H, D]),
                op0=mmax,
                op1=mult,
            )
            out_view = out[t * DT_ROWS : (t + 1) * DT_ROWS, :].rearrange(
                "(two p) hd -> p two hd", two=2
            )
            nc.sync.dma_start(out=out_view, in_=o_sb)

        for q in range(dtiles // QUAD):
            attA_slab = slabs.tile([8, QUAD, DT_ROWS], f32, tag="attA")
            src_slab = slabs.tile([8, 4 * QUAD, 1], f32, tag="src")
            lins = []
            for i in range(QUAD):
                t = q * QUAD + i
                if i % 2 == 0:
                    x2_sb = xin.tile([F, 2, DT_ROWS], f16, tag="x2")
                    nc.sync.dma_start(
                        out=x2_sb,
                        in_=xt[t : t + 2].rearrange("two f r -> f two r"),
                    )
                x_sl = x2_sb[:, i % 2, :]
                attT_ps, lin_ps = tile_front(t, x_sl)
                lins.append(lin_ps)
                # stage attA and the 4 src-term columns into the quad slabs
                nc.scalar.copy(out=attA_slab[:, i, :], in_=attT_ps[:, 0:DT_ROWS])
                src_view = attT_ps[:, DT_ROWS:].rearrange(
                    "h (b n) -> h b n", n=N
                )[:, :, 0:1]
                nc.scalar.copy(
                    out=src_slab[:, i * 4 : (i + 1) * 4, :], in_=src_view
                )

            # ---- batched attention chain over the quad ----
            nb = 4 * QUAD  # batch elements in the quad
            attS_slab = slabs.tile([8, nb, N], f32, tag="attS")
            nc.vector.tensor_tensor(
                out=attS_slab,
                in0=attA_slab.rearrange("h q r -> h (q r)").rearrange(
                    "h (b n) -> h b n", n=N
                ),
                in1=src_slab.to_broadcast([8, nb, N]),
                op=mybir.AluOpType.add,
            )
            attL_slab = slabs.tile([8, nb * N], f32, tag="attL")
            attL3 = attL_slab.rearrange("h (b n) -> h b n", n=N)
            nc.vector.tensor_scalar_mul(out=attL3, in0=attS_slab, scalar1=0.2)
            nc.vector.tensor_max(attL3, attL3, attS_slab)
            ew_slab = slabs.tile([8, nb * N], f32, tag="ew")
            nc.scalar.activation(
                out=ew_slab, in_=attL_slab, func=mybir.ActivationFunctionType.Exp
            )
            den_slab = slabs.tile([8, nb, 1], f32, tag="den")
            nc.vector.tensor_reduce(
                out=den_slab,
                in_=ew_slab.rearrange("h (b n) -> h b n", n=N),
                axis=mybir.AxisListType.X,
                op=mybir.AluOpType.add,
            )
            rden_slab = slabs.tile([8, nb, 1], f32, tag="rden")
            nc.vector.reciprocal(rden_slab, den_slab)
            awT_slab = slabs.tile([8, nb * N], f16, tag="awT")
            nc.vector.tensor_tensor(
                out=awT_slab.rearrange("h (b n) -> h b n", n=N),
                in0=ew_slab.rearrange("h (b n) -> h b n", n=N),
                in1=rden_slab.to_broadcast([8, nb, N]),
                op=mult,
            )

            for i in range(QUAD):
                tile_back(q * QUAD + i, lins[i], awT_slab, i)

    nc.compile()
    return nc


def _host_weights(W_lin, W_att):
    W_lin64 = W_lin.astype(np.float64)
    wc2 = (W_lin64 @ W_att[HD:].astype(np.float64)).astype(np.float32)
    wc1 = (W_lin64 @ W_att[:HD].astype(np.float64)).astype(np.float32)
    watt16 = np.ascontiguousarray(
        np.concatenate([wc2, wc1], axis=1).astype(np.float16)
    )
    ident8 = np.eye(8, dtype=np.float16)
    return W_lin.astype(np.float16), watt16, ident8


def _core_inputs(x_shard, mask_shard, wlin, watt16, ident8):
    nb = x_shard.shape[0]
    dtiles = nb * N // DT_ROWS
    xtv = np.ascontiguousarray(
        x_shard.reshape(dtiles, DT_ROWS, F).transpose(0, 2, 1).astype(np.float16)
    )
    mrow = np.where(mask_shard.reshape(1, -1) != 0, 0.0, -1e30).astype(
        ml_dtypes.bfloat16
    )
    return {
        "xt": xtv,
        "wlin": wlin,
        "watt": watt16,
        "ident8": ident8,
        "maskrow": mrow,
    }


def kernel(x, W_lin, W_att, mask):
    global LAST_RESULT
    x = np.asarray(x, dtype=np.float32)
    W_lin = np.asarray(W_lin, dtype=np.float32)
    W_att = np.asarray(W_att, dtype=np.float32)
    mask = np.asarray(mask)

    wlin, watt16, ident8 = _host_weights(W_lin, W_att)
    in_maps = []
    for c in range(NCORES):
        in_maps.append(
            _core_inputs(
                x[c * BSHARD : (c + 1) * BSHARD],
                mask[c * BSHARD : (c + 1) * BSHARD],
                wlin,
                watt16,
                ident8,
            )
        )

    nc = build_nc(DTILES)
    trace = os.environ.get("KERNEL_TRACE", "0") == "1"
    tmpdir = os.environ.get("KERNEL_TRACE_DIR") or None
    res = run_bass_kernel_spmd(
        nc, in_maps, list(range(NCORES)), trace=trace, tmpdir=tmpdir
    )
    LAST_RESULT = res
    return np.concatenate(
        [
            res.results[c]["out"].astype(np.float32).reshape(BSHARD, N, HD)
            for c in range(NCORES)
        ],
        axis=0,
    )


# revision 34
# speedup vs baseline: 4.0744x; 1.4288x over previous
"""Trainium2 Bass kernel for nn_AttentionAggregator (GNN message passing).

Math (per batch row b, with N=64 neighbors, F=128 in-features, H=8 heads, D=64):
    lin  = x @ W_lin                                      [B, N, 512]
    att  = lin[:,0,:] @ W_att[:512] + lin @ W_att[512:]   [B, N, 8]
    att  = LeakyReLU_0.2(att); masked softmax over N per (b, h)
    out  = relu(lin * aw)                                 [B, N, 512]

Design (v4, tuned against neuron-profile traces):
  * Attention contracts through W_lin (wc = W_lin @ W_att blocks) and is
    computed TRANSPOSED per 256-row tile: attT[16, 256] = watt16.T @ xT,
    so the softmax axis is a free dim (no cross-partition reductions).
    The mask is injected in LOGIT space pre-LeakyReLU via a rank-1 bf16
    matmul accumulate of {0,-1e30} rows (exp == 0 exactly, matching the
    reference's post-leaky -1e9).
  * fp16 matmul inputs (1 cyc/row on PE; fp32 runs ~4 cyc/row) and fp16
    output DMA (host upcasts) — halves both DMA directions. ~5e-4 rel err.
  * MEGA-tiling: 16 tiles form one mega. All PE front matmuls run first,
    then ONE set of attention-chain ops processes all 16 tiles at once
    with tiles packed 4-per-32-partition-group ([128, 1024] slabs instead
    of [8, 256] slivers — DVE cost scales with free size only), then the
    16 back-ends (aw transpose + fused relu*aw + store). This keeps the
    PE queue free of long-latency waits (back-to-back matmuls stay
    pipelined) and amortizes per-instruction overhead 16x.

Sharding: pure data-parallel over batch: 512 batch rows per core
(128 tiles of 256 rows), weights replicated.
"""

import os
from contextlib import ExitStack

import ml_dtypes
import numpy as np

import concourse.bacc as bacc
import concourse.bass as bass
import concourse.tile as tile
from concourse import mybir
from concourse.bass_utils import run_bass_kernel_spmd

B, N, F = 4096, 64, 128
H, D = 8, 64
HD = H * D  # 512
NCORES = 8
BSHARD = B // NCORES  # 512
ROWS = BSHARD * N  # 32768
DT_ROWS = 256  # rows per tile (4 batch elements)
DTILES = ROWS // DT_ROWS  # 128
MEGA = 16  # tiles per mega (4 partition groups x 4 slots)

f32 = mybir.dt.float32
bf16 = mybir.dt.bfloat16
f16 = mybir.dt.float16

LAST_RESULT = None  # test harness reads exec_time_ns / trace from here


def build_nc(dtiles: int = DTILES) -> bass.Bass:
    nc = bacc.Bacc("TRN2", target_bir_lowering=False, debug=False)
    rows = dtiles * DT_ROWS
    assert dtiles % MEGA == 0

    xt = nc.declare_dram_parameter("xt", [dtiles, F, DT_ROWS], f16, isOutput=False)
    wlin_d = nc.declare_dram_parameter("wlin", [F, HD], f16, isOutput=False)
    watt_d = nc.declare_dram_parameter("watt", [F, 16], f16, isOutput=False)
    ident_d = nc.declare_dram_parameter("ident8", [128, 8], f16, isOutput=False)
    maskrow_d = nc.declare_dram_parameter("maskrow", [1, rows], bf16, isOutput=False)
    out = nc.declare_dram_parameter("out", [rows, HD], f16, isOutput=True)

    mult = mybir.AluOpType.mult
    mmax = mybir.AluOpType.max

    with tile.TileContext(nc) as tc, ExitStack() as ctx:
        consts = ctx.enter_context(tc.tile_pool(name="consts", bufs=1))
        xin = ctx.enter_context(tc.tile_pool(name="xin", bufs=12))
        outp = ctx.enter_context(tc.tile_pool(name="outp", bufs=4))
        small = ctx.enter_context(tc.tile_pool(name="small", bufs=4))
        plin = ctx.enter_context(tc.tile_pool(name="plin", bufs=2, space="PSUM"))
        patt = ctx.enter_context(tc.tile_pool(name="patt", bufs=2, space="PSUM"))
        paw = ctx.enter_context(tc.tile_pool(name="paw", bufs=2, space="PSUM"))

        wlin_sb = consts.tile([F, HD], f16)
        nc.sync.dma_start(out=wlin_sb, in_=wlin_d[:])
        watt_sb = consts.tile([F, 16], f16)
        nc.sync.dma_start(out=watt_sb, in_=watt_d[:])
        # identity blocks replicated at partition bases 0/32/64/96 so the
        # aw transposes' fmap shares the weight operand's start partition
        ident_sb = consts.tile([128, 8], f16)
        nc.sync.dma_start(out=ident_sb, in_=ident_d[:])
        maskrow_sb = consts.tile([1, rows], bf16)
        nc.sync.dma_start(out=maskrow_sb, in_=maskrow_d[:])
        mones_sb = consts.tile([1, 8], bf16)
        nc.vector.memset(mones_sb, 1.0)

        # Persistent ping-pong mega slabs. Tiles pack 4-per-32-partition
        # group: tile i of a mega -> partition base 32*(i//4), free slot i%4.
        # memset once so untouched partitions stay finite for the sim.
        def mk_slabs(k):
            a = consts.tile([128, 4, DT_ROWS], f32, tag=f"slab_a{k}")
            s = consts.tile([128, 4, 4, 1], f32, tag=f"slab_s{k}")
            l = consts.tile([128, 4 * DT_ROWS], f32, tag=f"slab_l{k}")
            e = consts.tile([128, 4 * DT_ROWS], f32, tag=f"slab_e{k}")
            dn = consts.tile([128, 4, 4, 1], f32, tag=f"slab_dn{k}")
            rd = consts.tile([128, 4, 4, 1], f32, tag=f"slab_rd{k}")
            aw = consts.tile([128, 4 * DT_ROWS], f16, tag=f"slab_aw{k}")
            nc.vector.memset(a, 0.0)
            nc.vector.memset(s, 0.0)
            return a, s, l, e, dn, rd, aw

        slabs = [mk_slabs(0), mk_slabs(1)]

        for m in range(dtiles // MEGA):
            attA_m, src_m, attL_m, ew_m, den_m, rden_m, awT_m = slabs[m % 2]
            x_tiles = []
            # ---- fronts: attT matmuls + staging for 16 tiles (lin runs in
            # the backs so its PSUM lifetime stays within one tile) ----
            for i in range(MEGA):
                t = m * MEGA + i
                g, islot = i // 4, i % 4
                if i % 2 == 0:
                    x2_sb = xin.tile([F, 2, DT_ROWS], f16, tag="x2")
                    nc.sync.dma_start(
                        out=x2_sb,
                        in_=xt[t : t + 2].rearrange("two f r -> f two r"),
                    )
                x_sl = x2_sb[:, i % 2, :]
                x_tiles.append(x_sl)

                # attT[8, 512] = [attA cols 0:256 | attB cols 256:512],
                # all at partition base 0; mask accumulates onto attA.
                attT_ps = patt.tile([8, 2 * DT_ROWS], f32, tag="attT")
                nc.tensor.matmul(
                    attT_ps[:, 0:DT_ROWS],
                    watt_sb[:, 0:8],
                    x_sl,
                    start=True,
                    stop=False,
                )
                nc.tensor.matmul(
                    attT_ps[:, DT_ROWS:],
                    watt_sb[:, 8:16],
                    x_sl,
                    start=False,
                    stop=True,
                )
                nc.tensor.matmul(
                    attT_ps[:, 0:DT_ROWS],
                    mones_sb,
                    maskrow_sb[:, t * DT_ROWS : (t + 1) * DT_ROWS],
                    start=False,
                    stop=True,
                    skip_group_check=True,
                )
                nc.scalar.copy(
                    out=attA_m[32 * g : 32 * g + 8, islot, :],
                    in_=attT_ps[:, 0:DT_ROWS],
                )
                nc.scalar.copy(
                    out=src_m[32 * g : 32 * g + 8, islot, :, :],
                    in_=attT_ps[:, DT_ROWS:].rearrange("h (b n) -> h b n", n=N)[
                        :, :, 0:1
                    ],
                )

            # ---- one batched attention chain for the whole mega ----
            attS = attA_m.rearrange("p q (b n) -> p q b n", n=N)
            nc.vector.tensor_tensor(
                out=attS,
                in0=attS,
                in1=src_m.to_broadcast([128, 4, 4, N]),
                op=mybir.AluOpType.add,
            )
            nc.vector.scalar_tensor_tensor(
                out=attL_m.rearrange("p (q b n) -> p q b n", q=4, b=4),
                in0=attS,
                scalar=0.2,
                in1=attS,
                op0=mult,
                op1=mmax,
            )
            nc.scalar.activation(
                out=ew_m, in_=attL_m, func=mybir.ActivationFunctionType.Exp
            )
            nc.vector.tensor_reduce(
                out=den_m,
                in_=ew_m.rearrange("p (q b n) -> p q b n", q=4, b=4),
                axis=mybir.AxisListType.X,
                op=mybir.AluOpType.add,
            )
            nc.vector.reciprocal(rden_m, den_m)
            nc.vector.tensor_tensor(
                out=awT_m.rearrange("p (q b n) -> p q b n", q=4, b=4),
                in0=ew_m.rearrange("p (q b n) -> p q b n", q=4, b=4),
                in1=rden_m.to_broadcast([128, 4, 4, N]),
                op=mult,
            )

            # ---- backs: aw transpose + fused relu(lin)*aw + store ----
            for i in range(MEGA):
                t = m * MEGA + i
                g, islot = i // 4, i % 4
                lin_ps = plin.tile([128, 2, HD], f32, tag="lin")
                nc.tensor.matmul(
                    lin_ps[:, 0, :],
                    x_tiles[i][:, 0:128],
                    wlin_sb,
                    start=True,
                    stop=True,
                )
                nc.tensor.matmul(
                    lin_ps[:, 1, :],
                    x_tiles[i][:, 128:256],
                    wlin_sb,
                    start=True,
                    stop=True,
                )
                aw_ps = paw.tile([128, 16], f16, tag="aw_ps")
                for half in range(2):
                    nc.tensor.transpose(
                        aw_ps[:, half * 8 : half * 8 + 8],
                        awT_m[
                            32 * g : 32 * g + 8,
                            islot * DT_ROWS
                            + half * 128 : islot * DT_ROWS
                            + half * 128
                            + 128,
                        ],
                        ident_sb[32 * g : 32 * g + 8, :],
                        tile_position=(32 * g, 0),
                    )
                aw_sb = small.tile([128, 16], f16, tag="aw_sb")
                nc.scalar.copy(out=aw_sb, in_=aw_ps)

                o_sb = outp.tile([128, 2, HD], f16, tag="o")
                nc.vector.scalar_tensor_tensor(
                    out=o_sb.rearrange("p two (h d) -> p (two h) d", h=H),
                    in0=lin_ps.rearrange("p two (h d) -> p (two h) d", h=H),
                    scalar=0.0,
                    in1=aw_sb.to_broadcast([128, 2 * H, D]),
                    op0=mmax,
                    op1=mult,
                )
                out_view = out[t * DT_ROWS : (t + 1) * DT_ROWS, :].rearrange(
                    "(two p) hd -> p two hd", two=2
                )
                nc.sync.dma_start(out=out_view, in_=o_sb)

    nc.compile()
    return nc


def _host_weights(W_lin, W_att):
    W_lin64 = W_lin.astype(np.float64)
    wc2 = (W_lin64 @ W_att[HD:].astype(np.float64)).astype(np.float32)
    wc1 = (W_lin64 @ W_att[:HD].astype(np.float64)).astype(np.float32)
    watt16 = np.ascontiguousarray(
        np.concatenate([wc2, wc1], axis=1).astype(np.float16)
    )
    ident8 = np.zeros((128, 8), dtype=np.float16)
    for gg in range(4):
        ident8[32 * gg : 32 * gg + 8, :] = np.eye(8, dtype=np.float16)
    return W_lin.astype(np.float16), watt16, ident8


def _core_inputs(x_shard, mask_shard, wlin, watt16, ident8):
    nb = x_shard.shape[0]
    dtiles = nb * N // DT_ROWS
    xtv = np.ascontiguousarray(
        x_shard.reshape(dtiles, DT_ROWS, F).transpose(0, 2, 1).astype(np.float16)
    )
    mrow = np.where(mask_shard.reshape(1, -1) != 0, 0.0, -1e30).astype(
        ml_dtypes.bfloat16
    )
    return {
        "xt": xtv,
        "wlin": wlin,
        "watt": watt16,
        "ident8": ident8,
        "maskrow": mrow,
    }


def kernel(x, W_lin, W_att, mask):
    global LAST_RESULT
    x = np.asarray(x, dtype=np.float32)
    W_lin = np.asarray(W_lin, dtype=np.float32)
    W_att = np.asarray(W_att, dtype=np.float32)
    mask = np.asarray(mask)

    wlin, watt16, ident8 = _host_weights(W_lin, W_att)
    in_maps = []
    for c in range(NCORES):
        in_maps.append(
            _core_inputs(
                x[c * BSHARD : (c + 1) * BSHARD],
                mask[c * BSHARD : (c + 1) * BSHARD],
                wlin,
                watt16,
                ident8,
            )
        )

    nc = build_nc(DTILES)
    trace = os.environ.get("KERNEL_TRACE", "0") == "1"
    tmpdir = os.environ.get("KERNEL_TRACE_DIR") or None
    res = run_bass_kernel_spmd(
        nc, in_maps, list(range(NCORES)), trace=trace, tmpdir=tmpdir
    )
    LAST_RESULT = res
    return np.concatenate(
        [
            res.results[c]["out"].astype(np.float32).reshape(BSHARD, N, HD)
            for c in range(NCORES)
        ],
        axis=0,
    )


# revision 35
# speedup vs baseline: 4.2334x; 1.0390x over previous
"""Trainium2 Bass kernel for nn_AttentionAggregator (GNN message passing).

Math (per batch row b, with N=64 neighbors, F=128 in-features, H=8 heads, D=64):
    lin  = x @ W_lin                                      [B, N, 512]
    att  = lin[:,0,:] @ W_att[:512] + lin @ W_att[512:]   [B, N, 8]
    att  = LeakyReLU_0.2(att); masked softmax over N per (b, h)
    out  = relu(lin * aw)                                 [B, N, 512]

Design (v4, tuned against neuron-profile traces):
  * Attention contracts through W_lin (wc = W_lin @ W_att blocks) and is
    computed TRANSPOSED per 256-row tile: attT[16, 256] = watt16.T @ xT,
    so the softmax axis is a free dim (no cross-partition reductions).
    The mask is injected in LOGIT space pre-LeakyReLU via a rank-1 bf16
    matmul accumulate of {0,-1e30} rows (exp == 0 exactly, matching the
    reference's post-leaky -1e9).
  * fp16 matmul inputs (1 cyc/row on PE; fp32 runs ~4 cyc/row) and fp16
    output DMA (host upcasts) — halves both DMA directions. ~5e-4 rel err.
  * MEGA-tiling: 16 tiles form one mega. All PE front matmuls run first,
    then ONE set of attention-chain ops processes all 16 tiles at once
    with tiles packed 4-per-32-partition-group ([128, 1024] slabs instead
    of [8, 256] slivers — DVE cost scales with free size only), then the
    16 back-ends (aw transpose + fused relu*aw + store). This keeps the
    PE queue free of long-latency waits (back-to-back matmuls stay
    pipelined) and amortizes per-instruction overhead 16x.

Sharding: pure data-parallel over batch: 512 batch rows per core
(128 tiles of 256 rows), weights replicated.
"""

import os
from contextlib import ExitStack

import ml_dtypes
import numpy as np

import concourse.bacc as bacc
import concourse.bass as bass
import concourse.tile as tile
from concourse import mybir
from concourse.bass_utils import run_bass_kernel_spmd

B, N, F = 4096, 64, 128
H, D = 8, 64
HD = H * D  # 512
NCORES = 8
BSHARD = B // NCORES  # 512
ROWS = BSHARD * N  # 32768
DT_ROWS = 256  # rows per tile (4 batch elements)
DTILES = ROWS // DT_ROWS  # 128
MEGA = 16  # tiles per mega (4 partition groups x 4 slots)

f32 = mybir.dt.float32
bf16 = mybir.dt.bfloat16
f16 = mybir.dt.float16

LAST_RESULT = None  # test harness reads exec_time_ns / trace from here


def build_nc(dtiles: int = DTILES) -> bass.Bass:
    nc = bacc.Bacc("TRN2", target_bir_lowering=False, debug=False)
    rows = dtiles * DT_ROWS
    assert dtiles % MEGA == 0

    xt = nc.declare_dram_parameter("xt", [dtiles, F, DT_ROWS], f16, isOutput=False)
    wlin_d = nc.declare_dram_parameter("wlin", [F, HD], f16, isOutput=False)
    watt_d = nc.declare_dram_parameter("watt", [F, 16], f16, isOutput=False)
    ident_d = nc.declare_dram_parameter("ident8", [128, 8], f16, isOutput=False)
    maskrow_d = nc.declare_dram_parameter("maskrow", [1, rows], bf16, isOutput=False)
    out = nc.declare_dram_parameter("out", [rows, HD], f16, isOutput=True)

    mult = mybir.AluOpType.mult
    mmax = mybir.AluOpType.max

    with tile.TileContext(nc) as tc, ExitStack() as ctx:
        consts = ctx.enter_context(tc.tile_pool(name="consts", bufs=1))
        xin = ctx.enter_context(tc.tile_pool(name="xin", bufs=12))
        outp = ctx.enter_context(tc.tile_pool(name="outp", bufs=4))
        small = ctx.enter_context(tc.tile_pool(name="small", bufs=4))
        plin = ctx.enter_context(tc.tile_pool(name="plin", bufs=2, space="PSUM"))
        pattA = ctx.enter_context(tc.tile_pool(name="pattA", bufs=1, space="PSUM"))
        pattB = ctx.enter_context(tc.tile_pool(name="pattB", bufs=1, space="PSUM"))
        paw = ctx.enter_context(tc.tile_pool(name="paw", bufs=2, space="PSUM"))

        wlin_sb = consts.tile([F, HD], f16)
        nc.sync.dma_start(out=wlin_sb, in_=wlin_d[:])
        watt_sb = consts.tile([F, 16], f16)
        nc.sync.dma_start(out=watt_sb, in_=watt_d[:])
        # identity blocks replicated at partition bases 0/32/64/96 so the
        # aw transposes' fmap shares the weight operand's start partition
        ident_sb = consts.tile([128, 8], f16)
        nc.sync.dma_start(out=ident_sb, in_=ident_d[:])
        maskrow_sb = consts.tile([1, rows], bf16)
        nc.sync.dma_start(out=maskrow_sb, in_=maskrow_d[:])
        mones_sb = consts.tile([1, 8], bf16)
        nc.vector.memset(mones_sb, 1.0)

        # Persistent ping-pong mega slabs. Tiles pack 4-per-32-partition
        # group: tile i of a mega -> partition base 32*(i//4), free slot i%4.
        # memset once so untouched partitions stay finite for the sim.
        def mk_slabs(k):
            a = consts.tile([128, 4, DT_ROWS], f32, tag=f"slab_a{k}")
            s = consts.tile([128, 4, 4, 1], f32, tag=f"slab_s{k}")
            l = consts.tile([128, 4 * DT_ROWS], f32, tag=f"slab_l{k}")
            e = consts.tile([128, 4 * DT_ROWS], f32, tag=f"slab_e{k}")
            dn = consts.tile([128, 4, 4, 1], f32, tag=f"slab_dn{k}")
            rd = consts.tile([128, 4, 4, 1], f32, tag=f"slab_rd{k}")
            aw = consts.tile([128, 4 * DT_ROWS], f16, tag=f"slab_aw{k}")
            nc.vector.memset(a, 0.0)
            nc.vector.memset(s, 0.0)
            return a, s, l, e, dn, rd, aw

        slabs = [mk_slabs(0), mk_slabs(1)]

        for m in range(dtiles // MEGA):
            attA_m, src_m, attL_m, ew_m, den_m, rden_m, awT_m = slabs[m % 2]
            x_tiles = []
            # ---- fronts: pair-level attT matmuls (N=512) + staging; lin
            # runs in the backs so its PSUM lifetime stays within one tile ----
            for j in range(MEGA // 2):
                i = 2 * j
                t = m * MEGA + i
                g, islot = i // 4, i % 4
                x2_sb = xin.tile([F, 2, DT_ROWS], f16, tag="x2")
                nc.sync.dma_start(
                    out=x2_sb,
                    in_=xt[t : t + 2].rearrange("two f r -> f two r"),
                )
                x_tiles.append(x2_sb[:, 0, :])
                x_tiles.append(x2_sb[:, 1, :])
                x_pair = x2_sb.rearrange("f two r -> f (two r)")

                # attA for both tiles of the pair (one clean accumulate
                # group with the logit-space mask), attB in its own bank.
                attA_ps = pattA.tile([8, 2, DT_ROWS], f32, tag="attA")
                nc.tensor.matmul(
                    attA_ps.rearrange("h two r -> h (two r)"),
                    watt_sb[:, 0:8],
                    x_pair,
                    start=True,
                    stop=False,
                )
                nc.tensor.matmul(
                    attA_ps.rearrange("h two r -> h (two r)"),
                    mones_sb,
                    maskrow_sb[:, t * DT_ROWS : (t + 2) * DT_ROWS],
                    start=False,
                    stop=True,
                )
                attB_ps = pattB.tile([8, 2, DT_ROWS], f32, tag="attB")
                nc.tensor.matmul(
                    attB_ps.rearrange("h two r -> h (two r)"),
                    watt_sb[:, 8:16],
                    x_pair,
                    start=True,
                    stop=True,
                )
                nc.scalar.copy(
                    out=attA_m[32 * g : 32 * g + 8, islot : islot + 2, :],
                    in_=attA_ps,
                )
                nc.scalar.copy(
                    out=src_m[32 * g : 32 * g + 8, islot : islot + 2, :, :],
                    in_=attB_ps.rearrange("h two (b n) -> h two b n", n=N)[
                        :, :, :, 0:1
                    ],
                )

            # ---- one batched attention chain for the whole mega ----
            attS = attA_m.rearrange("p q (b n) -> p q b n", n=N)
            nc.vector.tensor_tensor(
                out=attS,
                in0=attS,
                in1=src_m.to_broadcast([128, 4, 4, N]),
                op=mybir.AluOpType.add,
            )
            nc.vector.scalar_tensor_tensor(
                out=attL_m.rearrange("p (q b n) -> p q b n", q=4, b=4),
                in0=attS,
                scalar=0.2,
                in1=attS,
                op0=mult,
                op1=mmax,
            )
            nc.scalar.activation(
                out=ew_m, in_=attL_m, func=mybir.ActivationFunctionType.Exp
            )
            nc.vector.tensor_reduce(
                out=den_m,
                in_=ew_m.rearrange("p (q b n) -> p q b n", q=4, b=4),
                axis=mybir.AxisListType.X,
                op=mybir.AluOpType.add,
            )
            nc.vector.reciprocal(rden_m, den_m)
            nc.vector.tensor_tensor(
                out=awT_m.rearrange("p (q b n) -> p q b n", q=4, b=4),
                in0=ew_m.rearrange("p (q b n) -> p q b n", q=4, b=4),
                in1=rden_m.to_broadcast([128, 4, 4, N]),
                op=mult,
            )

            # ---- backs: aw transpose + fused relu(lin)*aw + store ----
            for i in range(MEGA):
                t = m * MEGA + i
                g, islot = i // 4, i % 4
                if i % 2 == 0:
                    o2_sb = outp.tile([128, 2, 2, HD], f16, tag="o2")
                lin_ps = plin.tile([128, 2, HD], f32, tag="lin")
                nc.tensor.matmul(
                    lin_ps[:, 0, :],
                    x_tiles[i][:, 0:128],
                    wlin_sb,
                    start=True,
                    stop=True,
                )
                nc.tensor.matmul(
                    lin_ps[:, 1, :],
                    x_tiles[i][:, 128:256],
                    wlin_sb,
                    start=True,
                    stop=True,
                )
                aw_ps = paw.tile([128, 16], f16, tag="aw_ps")
                for half in range(2):
                    nc.tensor.transpose(
                        aw_ps[:, half * 8 : half * 8 + 8],
                        awT_m[
                            32 * g : 32 * g + 8,
                            islot * DT_ROWS
                            + half * 128 : islot * DT_ROWS
                            + half * 128
                            + 128,
                        ],
                        ident_sb[32 * g : 32 * g + 8, :],
                        tile_position=(32 * g, 0),
                    )
                aw_sb = small.tile([128, 16], f16, tag="aw_sb")
                nc.scalar.copy(out=aw_sb, in_=aw_ps)

                nc.vector.scalar_tensor_tensor(
                    out=o2_sb[:, i % 2].rearrange("p two (h d) -> p (two h) d", h=H),
                    in0=lin_ps.rearrange("p two (h d) -> p (two h) d", h=H),
                    scalar=0.0,
                    in1=aw_sb.to_broadcast([128, 2 * H, D]),
                    op0=mmax,
                    op1=mult,
                )
                if i % 2 == 1:
                    out_view = out[
                        (t - 1) * DT_ROWS : (t + 1) * DT_ROWS, :
                    ].rearrange("(four p) hd -> p four hd", four=4)
                    nc.sync.dma_start(
                        out=out_view,
                        in_=o2_sb.rearrange("p a b hd -> p (a b) hd"),
                    )

    nc.compile()
    return nc


def _host_weights(W_lin, W_att):
    W_lin64 = W_lin.astype(np.float64)
    wc2 = (W_lin64 @ W_att[HD:].astype(np.float64)).astype(np.float32)
    wc1 = (W_lin64 @ W_att[:HD].astype(np.float64)).astype(np.float32)
    watt16 = np.ascontiguousarray(
        np.concatenate([wc2, wc1], axis=1).astype(np.float16)
    )
    ident8 = np.zeros((128, 8), dtype=np.float16)
    for gg in range(4):
        ident8[32 * gg : 32 * gg + 8, :] = np.eye(8, dtype=np.float16)
    return W_lin.astype(np.float16), watt16, ident8


def _core_inputs(x_shard, mask_shard, wlin, watt16, ident8):
    nb = x_shard.shape[0]
    dtiles = nb * N // DT_ROWS
    xtv = np.ascontiguousarray(
        x_shard.reshape(dtiles, DT_ROWS, F).transpose(0, 2, 1).astype(np.float16)
    )
    mrow = np.where(mask_shard.reshape(1, -1) != 0, 0.0, -1e30).astype(
        ml_dtypes.bfloat16
    )
    return {
        "xt": xtv,
        "wlin": wlin,
        "watt": watt16,
        "ident8": ident8,
        "maskrow": mrow,
    }


def kernel(x, W_lin, W_att, mask):
    global LAST_RESULT
    x = np.asarray(x, dtype=np.float32)
    W_lin = np.asarray(W_lin, dtype=np.float32)
    W_att = np.asarray(W_att, dtype=np.float32)
    mask = np.asarray(mask)

    wlin, watt16, ident8 = _host_weights(W_lin, W_att)
    in_maps = []
    for c in range(NCORES):
        in_maps.append(
            _core_inputs(
                x[c * BSHARD : (c + 1) * BSHARD],
                mask[c * BSHARD : (c + 1) * BSHARD],
                wlin,
                watt16,
                ident8,
            )
        )

    nc = build_nc(DTILES)
    trace = os.environ.get("KERNEL_TRACE", "0") == "1"
    tmpdir = os.environ.get("KERNEL_TRACE_DIR") or None
    res = run_bass_kernel_spmd(
        nc, in_maps, list(range(NCORES)), trace=trace, tmpdir=tmpdir
    )
    LAST_RESULT = res
    return np.concatenate(
        [
            res.results[c]["out"].astype(np.float32).reshape(BSHARD, N, HD)
            for c in range(NCORES)
        ],
        axis=0,
    )


# revision 36
# speedup vs baseline: 4.5963x; 1.0857x over previous
"""Trainium2 Bass kernel for nn_AttentionAggregator (GNN message passing).

Math (per batch row b, with N=64 neighbors, F=128 in-features, H=8 heads, D=64):
    lin  = x @ W_lin                                      [B, N, 512]
    att  = lin[:,0,:] @ W_att[:512] + lin @ W_att[512:]   [B, N, 8]
    att  = LeakyReLU_0.2(att); masked softmax over N per (b, h)
    out  = relu(lin * aw)                                 [B, N, 512]

Design (v4, tuned against neuron-profile traces):
  * Attention contracts through W_lin (wc = W_lin @ W_att blocks) and is
    computed TRANSPOSED per 256-row tile: attT[16, 256] = watt16.T @ xT,
    so the softmax axis is a free dim (no cross-partition reductions).
    The mask is injected in LOGIT space pre-LeakyReLU via a rank-1 bf16
    matmul accumulate of {0,-1e30} rows (exp == 0 exactly, matching the
    reference's post-leaky -1e9).
  * fp16 matmul inputs (1 cyc/row on PE; fp32 runs ~4 cyc/row) and fp16
    output DMA (host upcasts) — halves both DMA directions. ~5e-4 rel err.
  * MEGA-tiling: 16 tiles form one mega. All PE front matmuls run first,
    then ONE set of attention-chain ops processes all 16 tiles at once
    with tiles packed 4-per-32-partition-group ([128, 1024] slabs instead
    of [8, 256] slivers — DVE cost scales with free size only), then the
    16 back-ends (aw transpose + fused relu*aw + store). This keeps the
    PE queue free of long-latency waits (back-to-back matmuls stay
    pipelined) and amortizes per-instruction overhead 16x.

Sharding: pure data-parallel over batch: 512 batch rows per core
(128 tiles of 256 rows), weights replicated.
"""

import os
from contextlib import ExitStack

import ml_dtypes
import numpy as np

import concourse.bacc as bacc
import concourse.bass as bass
import concourse.tile as tile
from concourse import mybir
from concourse.bass_utils import run_bass_kernel_spmd

B, N, F = 4096, 64, 128
H, D = 8, 64
HD = H * D  # 512
NCORES = 8
BSHARD = B // NCORES  # 512
ROWS = BSHARD * N  # 32768
DT_ROWS = 256  # rows per tile (4 batch elements)
DTILES = ROWS // DT_ROWS  # 128
MEGA = 16  # tiles per mega (4 partition groups x 4 slots)

f32 = mybir.dt.float32
bf16 = mybir.dt.bfloat16
f16 = mybir.dt.float16

LAST_RESULT = None  # test harness reads exec_time_ns / trace from here


def build_nc(dtiles: int = DTILES) -> bass.Bass:
    nc = bacc.Bacc("TRN2", target_bir_lowering=False, debug=False)
    rows = dtiles * DT_ROWS
    assert dtiles % MEGA == 0

    xt = nc.declare_dram_parameter("xt", [dtiles, F, DT_ROWS], f16, isOutput=False)
    wlin_d = nc.declare_dram_parameter("wlin", [F, HD], f16, isOutput=False)
    watt_d = nc.declare_dram_parameter("watt", [F, 16], f16, isOutput=False)
    ident_d = nc.declare_dram_parameter("ident8", [128, 8], f16, isOutput=False)
    maskrow_d = nc.declare_dram_parameter("maskrow", [1, rows], bf16, isOutput=False)
    out = nc.declare_dram_parameter("out", [rows, HD], f16, isOutput=True)

    mult = mybir.AluOpType.mult
    mmax = mybir.AluOpType.max

    with tile.TileContext(nc) as tc, ExitStack() as ctx:
        consts = ctx.enter_context(tc.tile_pool(name="consts", bufs=1))
        xin = ctx.enter_context(tc.tile_pool(name="xin", bufs=20))
        outp = ctx.enter_context(tc.tile_pool(name="outp", bufs=4))
        small = ctx.enter_context(tc.tile_pool(name="small", bufs=4))
        plin = ctx.enter_context(tc.tile_pool(name="plin", bufs=2, space="PSUM"))
        pattA = ctx.enter_context(tc.tile_pool(name="pattA", bufs=1, space="PSUM"))
        pattB = ctx.enter_context(tc.tile_pool(name="pattB", bufs=1, space="PSUM"))
        paw = ctx.enter_context(tc.tile_pool(name="paw", bufs=2, space="PSUM"))

        wlin_sb = consts.tile([F, HD], f16)
        nc.sync.dma_start(out=wlin_sb, in_=wlin_d[:])
        watt_sb = consts.tile([F, 16], f16)
        nc.sync.dma_start(out=watt_sb, in_=watt_d[:])
        # identity blocks replicated at partition bases 0/32/64/96 so the
        # aw transposes' fmap shares the weight operand's start partition
        ident_sb = consts.tile([128, 8], f16)
        nc.sync.dma_start(out=ident_sb, in_=ident_d[:])
        maskrow_sb = consts.tile([1, rows], bf16)
        nc.sync.dma_start(out=maskrow_sb, in_=maskrow_d[:])
        mones_sb = consts.tile([1, 8], bf16)
        nc.vector.memset(mones_sb, 1.0)

        # Persistent ping-pong mega slabs. Tiles pack 4-per-32-partition
        # group: tile i of a mega -> partition base 32*(i//4), free slot i%4.
        # memset once so untouched partitions stay finite for the sim.
        def mk_slabs(k):
            a = consts.tile([128, 4, DT_ROWS], f32, tag=f"slab_a{k}")
            s = consts.tile([128, 4, 4, 1], f32, tag=f"slab_s{k}")
            l = consts.tile([128, 4 * DT_ROWS], f32, tag=f"slab_l{k}")
            e = consts.tile([128, 4 * DT_ROWS], f32, tag=f"slab_e{k}")
            dn = consts.tile([128, 4, 4, 1], f32, tag=f"slab_dn{k}")
            rd = consts.tile([128, 4, 4, 1], f32, tag=f"slab_rd{k}")
            aw = consts.tile([128, 4 * DT_ROWS], f16, tag=f"slab_aw{k}")
            nc.vector.memset(a, 0.0)
            nc.vector.memset(s, 0.0)
            return a, s, l, e, dn, rd, aw

        slabs = [mk_slabs(0), mk_slabs(1)]

        def fronts(m):
            attA_m, src_m, attL_m, ew_m, den_m, rden_m, awT_m = slabs[m % 2]
            x_tiles = []
            # pair-level attT matmuls (N=512) + staging; lin runs in the
            # backs so its PSUM lifetime stays within one tile
            for j in range(MEGA // 2):
                i = 2 * j
                t = m * MEGA + i
                g, islot = i // 4, i % 4
                x2_sb = xin.tile([F, 2, DT_ROWS], f16, tag="x2")
                nc.sync.dma_start(
                    out=x2_sb,
                    in_=xt[t : t + 2].rearrange("two f r -> f two r"),
                )
                x_tiles.append(x2_sb[:, 0, :])
                x_tiles.append(x2_sb[:, 1, :])
                x_pair = x2_sb.rearrange("f two r -> f (two r)")

                # attA for both tiles of the pair (one clean accumulate
                # group with the logit-space mask), attB in its own bank.
                attA_ps = pattA.tile([8, 2, DT_ROWS], f32, tag="attA")
                nc.tensor.matmul(
                    attA_ps.rearrange("h two r -> h (two r)"),
                    watt_sb[:, 0:8],
                    x_pair,
                    start=True,
                    stop=False,
                )
                nc.tensor.matmul(
                    attA_ps.rearrange("h two r -> h (two r)"),
                    mones_sb,
                    maskrow_sb[:, t * DT_ROWS : (t + 2) * DT_ROWS],
                    start=False,
                    stop=True,
                )
                attB_ps = pattB.tile([8, 2, DT_ROWS], f32, tag="attB")
                nc.tensor.matmul(
                    attB_ps.rearrange("h two r -> h (two r)"),
                    watt_sb[:, 8:16],
                    x_pair,
                    start=True,
                    stop=True,
                )
                nc.scalar.copy(
                    out=attA_m[32 * g : 32 * g + 8, islot : islot + 2, :],
                    in_=attA_ps,
                )
                nc.scalar.copy(
                    out=src_m[32 * g : 32 * g + 8, islot : islot + 2, :, :],
                    in_=attB_ps.rearrange("h two (b n) -> h two b n", n=N)[
                        :, :, :, 0:1
                    ],
                )

            return x_tiles

        def chain(m):
            attA_m, src_m, attL_m, ew_m, den_m, rden_m, awT_m = slabs[m % 2]
            # one batched attention chain for the whole mega
            attS = attA_m.rearrange("p q (b n) -> p q b n", n=N)
            nc.vector.tensor_tensor(
                out=attS,
                in0=attS,
                in1=src_m.to_broadcast([128, 4, 4, N]),
                op=mybir.AluOpType.add,
            )
            nc.vector.scalar_tensor_tensor(
                out=attL_m.rearrange("p (q b n) -> p q b n", q=4, b=4),
                in0=attS,
                scalar=0.2,
                in1=attS,
                op0=mult,
                op1=mmax,
            )
            nc.scalar.activation(
                out=ew_m, in_=attL_m, func=mybir.ActivationFunctionType.Exp
            )
            nc.vector.tensor_reduce(
                out=den_m,
                in_=ew_m.rearrange("p (q b n) -> p q b n", q=4, b=4),
                axis=mybir.AxisListType.X,
                op=mybir.AluOpType.add,
            )
            nc.vector.reciprocal(rden_m, den_m)
            nc.vector.tensor_tensor(
                out=awT_m.rearrange("p (q b n) -> p q b n", q=4, b=4),
                in0=ew_m.rearrange("p (q b n) -> p q b n", q=4, b=4),
                in1=rden_m.to_broadcast([128, 4, 4, N]),
                op=mult,
            )

        def backs(m, x_tiles):
            attA_m, src_m, attL_m, ew_m, den_m, rden_m, awT_m = slabs[m % 2]
            # aw transpose + fused relu(lin)*aw + store
            for i in range(MEGA):
                t = m * MEGA + i
                g, islot = i // 4, i % 4
                if i % 2 == 0:
                    o2_sb = outp.tile([128, 2, 2, HD], f16, tag="o2")
                lin_ps = plin.tile([128, 2, HD], f32, tag="lin")
                nc.tensor.matmul(
                    lin_ps[:, 0, :],
                    x_tiles[i][:, 0:128],
                    wlin_sb,
                    start=True,
                    stop=True,
                )
                nc.tensor.matmul(
                    lin_ps[:, 1, :],
                    x_tiles[i][:, 128:256],
                    wlin_sb,
                    start=True,
                    stop=True,
                )
                aw_ps = paw.tile([128, 16], f16, tag="aw_ps")
                for half in range(2):
                    nc.tensor.transpose(
                        aw_ps[:, half * 8 : half * 8 + 8],
                        awT_m[
                            32 * g : 32 * g + 8,
                            islot * DT_ROWS
                            + half * 128 : islot * DT_ROWS
                            + half * 128
                            + 128,
                        ],
                        ident_sb[32 * g : 32 * g + 8, :],
                        tile_position=(32 * g, 0),
                    )
                aw_sb = small.tile([128, 16], f16, tag="aw_sb")
                nc.scalar.copy(out=aw_sb, in_=aw_ps)

                nc.vector.scalar_tensor_tensor(
                    out=o2_sb[:, i % 2].rearrange("p two (h d) -> p (two h) d", h=H),
                    in0=lin_ps.rearrange("p two (h d) -> p (two h) d", h=H),
                    scalar=0.0,
                    in1=aw_sb.to_broadcast([128, 2 * H, D]),
                    op0=mmax,
                    op1=mult,
                )
                if i % 2 == 1:
                    out_view = out[
                        (t - 1) * DT_ROWS : (t + 1) * DT_ROWS, :
                    ].rearrange("(four p) hd -> p four hd", four=4)
                    nc.sync.dma_start(
                        out=out_view,
                        in_=o2_sb.rearrange("p a b hd -> p (a b) hd"),
                    )

        # software-pipelined mega order: PE runs fronts(m+1) while the
        # DVE/ACT chain of mega m drains, then the backs of mega m
        nmega = dtiles // MEGA
        xt_prev = fronts(0)
        for m in range(nmega):
            chain(m)
            xt_next = fronts(m + 1) if m + 1 < nmega else None
            backs(m, xt_prev)
            xt_prev = xt_next

    nc.compile()
    return nc


def _host_weights(W_lin, W_att):
    W_lin64 = W_lin.astype(np.float64)
    wc2 = (W_lin64 @ W_att[HD:].astype(np.float64)).astype(np.float32)
    wc1 = (W_lin64 @ W_att[:HD].astype(np.float64)).astype(np.float32)
    watt16 = np.ascontiguousarray(
        np.concatenate([wc2, wc1], axis=1).astype(np.float16)
    )
    ident8 = np.zeros((128, 8), dtype=np.float16)
    for gg in range(4):
        ident8[32 * gg : 32 * gg + 8, :] = np.eye(8, dtype=np.float16)
    return W_lin.astype(np.float16), watt16, ident8


def _core_inputs(x_shard, mask_shard, wlin, watt16, ident8):
    nb = x_shard.shape[0]
    dtiles = nb * N // DT_ROWS
    xtv = np.ascontiguousarray(
        x_shard.reshape(dtiles, DT_ROWS, F).transpose(0, 2, 1).astype(np.float16)
    )
    mrow = np.where(mask_shard.reshape(1, -1) != 0, 0.0, -1e30).astype(
        ml_dtypes.bfloat16
    )
    return {
        "xt": xtv,
        "wlin": wlin,
        "watt": watt16,
        "ident8": ident8,
        "maskrow": mrow,
    }


def kernel(x, W_lin, W_att, mask):
    global LAST_RESULT
    x = np.asarray(x, dtype=np.float32)
    W_lin = np.asarray(W_lin, dtype=np.float32)
    W_att = np.asarray(W_att, dtype=np.float32)
    mask = np.asarray(mask)

    wlin, watt16, ident8 = _host_weights(W_lin, W_att)
    in_maps = []
    for c in range(NCORES):
        in_maps.append(
            _core_inputs(
                x[c * BSHARD : (c + 1) * BSHARD],
                mask[c * BSHARD : (c + 1) * BSHARD],
                wlin,
                watt16,
                ident8,
            )
        )

    nc = build_nc(DTILES)
    trace = os.environ.get("KERNEL_TRACE", "0") == "1"
    tmpdir = os.environ.get("KERNEL_TRACE_DIR") or None
    res = run_bass_kernel_spmd(
        nc, in_maps, list(range(NCORES)), trace=trace, tmpdir=tmpdir
    )
    LAST_RESULT = res
    return np.concatenate(
        [
            res.results[c]["out"].astype(np.float32).reshape(BSHARD, N, HD)
            for c in range(NCORES)
        ],
        axis=0,
    )
